# revision 13
# baseline (speedup 1.0000x reference)
"""Causal self-attention (B=1, T=4096, C=768, H=12, D=64) on 8 NeuronCores.

Tensor-parallel, 1.5 heads per core: core c owns full head c ("A") plus
half of shared head 8+c//2 ("B") — the even (c%2==0) or odd k-tiles of
every supertile, so both halves have identical program shape (uniform
SPMD).  The host merges the B halves: out += (rawWp_i + rawWp_j) /
(den_i + den_j), which is exact because the per-q softmax division
commutes with the right-multiplication by w_proj.

Per core, software-pipelined:
  phase 1 (8 steps, x^T + parity-packed xB^T streamed through SBUF rings):
    P  = [qA|qB]^T x^T + b      [128,T]   (rows 0:64 qA*0.125, 64:128 qB)
    K' = [kA|vA]^T x^T + b      [128,512] (kA rows persist; vA rows
                                 PE-transposed into v slots [T,64]+ones)
    Bb = [vB|kB]^T xB^T + b     [128,256] (kB rows persist @64:128, so the
                                 s^T matmul bases match qB; vB transposed)
  phase 2 (A(J) and B(J) generators interleaved):
    s^T[k,q] = k.q in PSUM [128,1024] chunks (2 k-tiles); causal masks are
       additive bf16 matmuls — static tri for A, per-core mask data for B
       (B's diagonal offset depends on the core's k-parity)
    p^T = exp(s^T) on ScalarE -> SBUF
    yt[65,q] = [v|1]^T p^T accumulated in PSUM (row 64 = denominators)
    A: recip(den) on DVE, PE-transpose [1,128]->[128,1] per q-tile, then
       normalization rides the proj PSUM->SBUF move as tensor_scalar mult
    B: no normalization — raw yt + den row go out; host divides
  proj per q-tile: outA[q,768] = yA^T WpA (scaled by 1/denA),
                   outB[q,768] = yB_raw^T WpB (raw), both DMA'd out

All matmul operands float32r (full PE rate at N>=256, HW-verified
213ns per 512-col matmul).
"""
import numpy as np
from contextlib import ExitStack

import concourse.bass as bass
import concourse.mybir as mybir
import concourse.tile as tile
from concourse import bacc
from concourse.bass import ts
from concourse.bass_utils import run_bass_kernel_spmd

try:
    import ml_dtypes
    ml_bf16 = ml_dtypes.bfloat16
except ImportError:  # pragma: no cover
    ml_bf16 = np.float32

F32 = mybir.dt.float32
F32R = mybir.dt.float32r
BF16 = mybir.dt.bfloat16
EXP = mybir.ActivationFunctionType.Exp

T, C, H, D = 4096, 768, 12, 64
KC = C // 128          # 6 contraction chunks of 128
TQ = 512               # q supertile width
NJ = T // TQ           # 8 supertiles
NT = T // 128          # 32 k-tiles (head A); head B has NT//2 packed tiles
NTB = NT // 2
TB = T // 2            # 2048 packed B columns
CH = 2                 # k-tiles per exp chunk
NEG = -60.0            # additive mask value (exp(-60) ~ 0)
NSLOT = NT + NTB       # v slots: A tiles then B tiles

_CACHE = {}


def build_program(reps=1):
    nc = bacc.Bacc()
    xT = nc.dram_tensor("xT", [C, T], F32R, kind="ExternalInput")
    xbT = nc.dram_tensor("xbT", [C, TB], F32R, kind="ExternalInput")
    # weight blocks: [block P | block K' | block Bb] along last axis
    wblk = nc.dram_tensor("wblk", [3, C, 128], F32R, kind="ExternalInput")
    bblk = nc.dram_tensor("bblk", [3, 128], F32, kind="ExternalInput")
    wp = nc.dram_tensor("wp", [128, C], F32R, kind="ExternalInput")
    tri = nc.dram_tensor("tri", [128, 128], BF16, kind="ExternalInput")
    trib1 = nc.dram_tensor("trib1", [128, 256], BF16, kind="ExternalInput")
    trib2 = nc.dram_tensor("trib2", [128, 512], BF16, kind="ExternalInput")
    identb = nc.dram_tensor("identb", [128, 128], BF16, kind="ExternalInput")
    ident = nc.dram_tensor("ident", [128, 128], F32R, kind="ExternalInput")
    onesd = nc.dram_tensor("onesd", [NSLOT * 2], F32R, kind="ExternalInput")
    outa = nc.dram_tensor("outa", [T, C], F32, kind="ExternalOutput")
    outb = nc.dram_tensor("outb", [T, C], F32, kind="ExternalOutput")
    denb = nc.dram_tensor("denb", [NJ, TQ], F32, kind="ExternalOutput")

    with ExitStack() as ctx:
        tc = ctx.enter_context(tile.TileContext(nc))
        singles = ctx.enter_context(tc.tile_pool(name="singles", bufs=1))
        ring = ctx.enter_context(tc.tile_pool(name="ring", bufs=12))
        ringb = ctx.enter_context(tc.tile_pool(name="ringb", bufs=12))
        vring = ctx.enter_context(tc.tile_pool(name="vring", bufs=4))
        sb_p = ctx.enter_context(tc.tile_pool(name="sb_p", bufs=4))
        sb_y = ctx.enter_context(tc.tile_pool(name="sb_y", bufs=2))
        sb_r = ctx.enter_context(tc.tile_pool(name="sb_r", bufs=2))
        sb_o = ctx.enter_context(tc.tile_pool(name="sb_o", bufs=3))
        ps_qk = ctx.enter_context(tc.tile_pool(name="ps_qk", bufs=2, space="PSUM"))
        ps_s = ctx.enter_context(tc.tile_pool(name="ps_s", bufs=2, space="PSUM"))
        ps_yt = ctx.enter_context(tc.tile_pool(name="ps_yt", bufs=2, space="PSUM"))

        # ---- constants / weights ----
        wblk_sb = singles.tile([128, 3, KC, 128], F32R)
        nc.sync.dma_start(
            wblk_sb, wblk.rearrange("b (kc p) m -> p b kc m", p=128))
        bblk_sb = singles.tile([128, 3], F32)
        nc.sync.dma_start(bblk_sb, bblk.rearrange("b p -> p b"))
        wp_sb = singles.tile([128, C], F32R)
        tri_sb = singles.tile([128, 128], BF16)
        nc.sync.dma_start(tri_sb, tri[:, :])
        trib1_sb = singles.tile([128, 256], BF16)
        nc.sync.dma_start(trib1_sb, trib1[:, :])
        trib2_sb = singles.tile([128, 512], BF16)
        nc.sync.dma_start(trib2_sb, trib2[:, :])
        identb_sb = singles.tile([128, 128], BF16)
        nc.sync.dma_start(identb_sb, identb[:, :])
        ident_sb = singles.tile([128, 128], F32R)
        nc.sync.dma_start(ident_sb, ident[:, :])
        v_sb = singles.tile([128, NSLOT * 66], F32R)

        # persistent state
        qkP = singles.tile([128, T], F32R)    # rows 0:64 qA, 64:128 qB
        kA = singles.tile([128, T], F32R)     # rows 0:64 kA (64:128 unused)
        kB = singles.tile([128, TB], F32R)    # rows 64:128 kB (packed)
        yfin = singles.tile([128, T], F32R)   # 0:64 yA norm, 64:128 yB raw
        recden = singles.tile([128, NJ * 4], F32)  # 1/denA cols per q-tile

        def vslot(i):
            return i * 66

        def qkv_step(tc_i):
            """Stream x/xB column slices; compute P,K',Bb; transpose v."""
            xs, xbs = [], []
            for kc in range(KC):
                x_sl = ring.tile([128, TQ], F32R, tag="xr", name="x_sl")
                nc.sync.dma_start(x_sl, xT[ts(kc, 128), ts(tc_i, TQ)])
                xs.append(x_sl)
                xb_sl = ringb.tile([128, 256], F32R, tag="xbr", name="xb_sl")
                nc.sync.dma_start(xb_sl, xbT[ts(kc, 128), ts(tc_i, 256)])
                xbs.append(xb_sl)
            # P block
            psP = ps_qk.tile([128, TQ], F32, tag="qk", name="psP")
            for kc in range(KC):
                nc.tensor.matmul(
                    psP, lhsT=wblk_sb[:, 0, kc, :], rhs=xs[kc],
                    start=(kc == 0), stop=(kc == KC - 1))
            nc.vector.tensor_scalar_add(
                qkP[:, ts(tc_i, TQ)], psP, bblk_sb[:, 0:1])
            # K' block (kA | vA)
            psK = ps_qk.tile([128, TQ], F32, tag="qk", name="psK")
            for kc in range(KC):
                nc.tensor.matmul(
                    psK, lhsT=wblk_sb[:, 1, kc, :], rhs=xs[kc],
                    start=(kc == 0), stop=(kc == KC - 1))
            nc.vector.tensor_scalar_add(
                kA[:, ts(tc_i, TQ)], psK, bblk_sb[:, 1:2])
            # Bb block (vB | kB), 256 cols
            psB = ps_qk.tile([128, 256], F32, tag="qk", name="psB")
            for kc in range(KC):
                nc.tensor.matmul(
                    psB, lhsT=wblk_sb[:, 2, kc, :], rhs=xbs[kc],
                    start=(kc == 0), stop=(kc == KC - 1))
            vbt = vring.tile([128, 256], F32R, tag="vbt", name="vbt")
            nc.vector.tensor_scalar_add(vbt, psB, bblk_sb[:, 2:3])
            nc.sync.dma_start(kB[64:128, ts(tc_i, 256)], vbt[64:128, :])
            # vA transposes: global tiles 4t..4t+3 from kA rows 64:128
            for il in range(4):
                i = 4 * tc_i + il
                tp = ps_qk.tile([128, 64], F32R, tag="qk", name="tpA")
                nc.tensor.transpose(
                    tp, kA[64:128, tc_i * TQ + il * 128 :
                           tc_i * TQ + (il + 1) * 128],
                    ident_sb[64:128, 64:128])
                nc.vector.tensor_copy(
                    v_sb[:, vslot(i) : vslot(i) + 64], tp)
            # vB transposes: packed tiles 2t, 2t+1 from vbt rows 0:64
            for il in range(2):
                m = 2 * tc_i + il
                tp2 = ps_qk.tile([128, 64], F32R, tag="qk", name="tpB")
                nc.tensor.transpose(
                    tp2, vbt[0:64, ts(il, 128)], ident_sb[0:64, 0:64])
                nc.vector.tensor_copy(
                    v_sb[:, vslot(NT + m) : vslot(NT + m) + 64], tp2)

        def att_gen(h, J):
            """h=0: full head A over 4J+4 k-tiles; h=1: parity half head B
            over 2J+2 packed tiles."""
            nkt = 4 * J + 4 if h == 0 else 2 * J + 2
            chunks = [list(range(nkt))[i : i + CH] for i in range(0, nkt, CH)]
            yt = ps_yt.tile([128, TQ], F32, tag="yt", name="yt")
            state = {"first": True}

            def emit_s(ch_tiles):
                st = ps_s.tile([128, CH * TQ], F32, tag="st", name="st")
                for j, i in enumerate(ch_tiles):
                    if h == 0:
                        d = i - 4 * J
                        nc.tensor.matmul(
                            st[:, ts(j, TQ)], lhsT=kA[0:64, ts(i, 128)],
                            rhs=qkP[0:64, ts(J, TQ)],
                            start=True, stop=(d < 0))
                        if d >= 0:
                            nc.tensor.matmul(
                                st[:, j * TQ + d * 128 : j * TQ + (d + 1) * 128],
                                lhsT=tri_sb, rhs=identb_sb,
                                start=False, stop=True, skip_group_check=True)
                    else:
                        diag = i >= 2 * J
                        nc.tensor.matmul(
                            st[:, ts(j, TQ)], lhsT=kB[64:128, ts(i, 128)],
                            rhs=qkP[64:128, ts(J, TQ)],
                            start=True, stop=not diag)
                        if diag:
                            # local tile 2J -> mask cols 0:256 (trib1),
                            # local tile 2J+1 -> mask cols 0:512 (trib2)
                            if i == 2 * J:
                                nc.tensor.matmul(
                                    st[:, j * TQ : j * TQ + 256],
                                    lhsT=identb_sb, rhs=trib1_sb,
                                    start=False, stop=True,
                                    skip_group_check=True)
                            else:
                                nc.tensor.matmul(
                                    st[:, j * TQ : j * TQ + 512],
                                    lhsT=identb_sb, rhs=trib2_sb,
                                    start=False, stop=True,
                                    skip_group_check=True)
                pt = sb_p.tile([128, CH * TQ], F32R, tag="pt", name="pt")
                n = len(ch_tiles) * TQ
                nc.scalar.activation(pt[:, :n], st[:, :n], EXP)
                return pt

            def emit_pv(ch_tiles, pt):
                for j, i in enumerate(ch_tiles):
                    slot = i if h == 0 else NT + i
                    if h == 0:
                        d = i - 4 * J
                        q0 = d * 128 if d > 0 else 0
                    else:
                        q0 = 0
                    nc.tensor.matmul(
                        yt[0:66, q0:TQ],
                        lhsT=v_sb[:, vslot(slot) : vslot(slot) + 66],
                        rhs=pt[:, j * TQ + q0 : (j + 1) * TQ],
                        start=state["first"], stop=(i == nkt - 1),
                        skip_group_check=True)
                    state["first"] = False

            pts = []
            for ci in range(len(chunks) + 1):
                if ci < len(chunks):
                    pts.append(emit_s(chunks[ci]))
                if ci >= 1:
                    emit_pv(chunks[ci - 1], pts[ci - 1])
                yield

            if h == 0:
                # yA + 1/denA: recip the den row, PE-transpose per q-tile
                # (2-row block: fp32r matmul dst innermost count must be even)
                nc.vector.tensor_copy(yfin[0:64, ts(J, TQ)], yt[0:64])
                dr = sb_r.tile([2, TQ], F32R, tag="dr", name="dr")
                with nc.allow_low_precision(reason="fp32r for PE"):
                    nc.vector.reciprocal(dr, yt[64:66, :])
                for qt in range(4):
                    tpd = ps_qk.tile([128, 64], F32R, tag="qk", name="tpd")
                    nc.tensor.transpose(
                        tpd[:, 0:2], dr[0:2, ts(qt, 128)], ident_sb[0:2, 0:2])
                    nc.vector.tensor_copy(
                        recden[:, J * 4 + qt : J * 4 + qt + 1], tpd[:, 0:1])
            else:
                # raw yB + raw den out
                nc.vector.tensor_copy(yfin[64:128, ts(J, TQ)], yt[0:64])
                db = sb_r.tile([1, TQ], F32, tag="db", name="db")
                nc.vector.tensor_copy(db, yt[64:65, :])
                nc.sync.dma_start(denb[J, :], db[0:1, :])

        def proj_step(J):
            for qt in range(4):
                q0 = J * TQ + qt * 128
                rd = recden[:, J * 4 + qt : J * 4 + qt + 1]
                oba = sb_o.tile([128, C], F32, tag="oba", name="oba")
                obb = sb_o.tile([128, C], F32, tag="obb", name="obb")
                for (lo, hi) in ((0, 512), (512, 768)):
                    ppa = ps_qk.tile([128, hi - lo], F32, tag="qk", name="ppa")
                    nc.tensor.matmul(ppa, lhsT=yfin[0:64, q0 : q0 + 128],
                                     rhs=wp_sb[0:64, lo:hi],
                                     start=True, stop=True)
                    nc.vector.tensor_scalar_mul(oba[:, lo:hi], ppa, rd)
                    ppb = ps_qk.tile([128, hi - lo], F32, tag="qk", name="ppb")
                    nc.tensor.matmul(ppb, lhsT=yfin[64:128, q0 : q0 + 128],
                                     rhs=wp_sb[64:128, lo:hi],
                                     start=True, stop=True)
                    nc.vector.tensor_copy(obb[:, lo:hi], ppb)
                nc.sync.dma_start(outa[q0 : q0 + 128, :], oba)
                nc.sync.dma_start(outb[q0 : q0 + 128, :], obb)

        def drive(*gens):
            gl = list(gens)
            while gl:
                for g in list(gl):
                    try:
                        next(g)
                    except StopIteration:
                        gl.remove(g)

        for rep in range(reps):
            a_done, b_done, projected = set(), set(), set()

            def flush_proj():
                for J in range(NJ):
                    if J in a_done and J in b_done and J not in projected:
                        proj_step(J)
                        projected.add(J)

            for t in range(NJ):
                qkv_step(t)
                if t == 0 and rep == 0:
                    nc.sync.dma_start(wp_sb, wp[:, :])
                    ones_view = bass.AP(
                        tensor=v_sb.tensor, offset=v_sb.offset + 64,
                        ap=[list(p) for p in v_sb.ap[:1]] + [[66, NSLOT],
                                                             [1, 2]])
                    nc.sync.dma_start(
                        ones_view,
                        onesd[:][0 : NSLOT * 2].partition_broadcast(128))
                if 1 <= t <= 6:            # A(0..2)/B(0..2) ride phase 1
                    if t % 2 == 1:
                        drive(att_gen(0, t // 2))
                        a_done.add(t // 2)
                    else:
                        drive(att_gen(1, t // 2 - 1))
                        b_done.add(t // 2 - 1)
            for s in range(5):
                drive(att_gen(0, s + 3), att_gen(1, s + 3))
                a_done.add(s + 3)
                b_done.add(s + 3)
                flush_proj()
            flush_proj()

    if not nc.is_finalized():
        nc.finalize()
    return nc


def _make_inputs(x, w_attn, b_attn, w_proj):
    """Build the 8 per-core input maps from full inputs."""
    xTc = np.ascontiguousarray(x.reshape(T, C).T).astype(np.float32)
    tri_np = np.where(np.arange(128)[:, None] >= np.arange(128)[None, :],
                      0.0, NEG).astype(ml_bf16)
    identb_np = np.eye(128, dtype=np.float32).astype(ml_bf16)
    ident_np = np.eye(128, dtype=np.float32)
    onesd_np = np.ones((NSLOT * 2,), np.float32)

    def mask_block(d, width):
        """[128, width] additive mask for a diag k-tile at column offset d."""
        m = np.zeros((128, width), np.float32)
        m[:, : d * 128] = NEG
        if d * 128 < width:
            # applied untransposed (rhs operand): keep k_row <= q_col
            blk = m[:, d * 128 : (d + 1) * 128]
            blk[:] = np.where(
                np.arange(128)[:, None] <= np.arange(128)[None, :], 0.0, NEG)
        return m.astype(ml_bf16)

    in_maps = []
    for c in range(8):
        hA = c
        hB = 8 + c // 2
        delta = c % 2
        qa, ka, va = hA * 64, C + hA * 64, 2 * C + hA * 64
        qb, kb_, vb = hB * 64, C + hB * 64, 2 * C + hB * 64

        wblk_np = np.zeros((3, C, 128), np.float32)
        bblk_np = np.zeros((3, 128), np.float32)
        # P = [qA*0.125 | qB*0.125]
        wblk_np[0, :, 0:64] = w_attn[:, qa : qa + 64] * 0.125
        wblk_np[0, :, 64:128] = w_attn[:, qb : qb + 64] * 0.125
        bblk_np[0, 0:64] = b_attn[qa : qa + 64] * 0.125
        bblk_np[0, 64:128] = b_attn[qb : qb + 64] * 0.125
        # K' = [kA | vA]
        wblk_np[1, :, 0:64] = w_attn[:, ka : ka + 64]
        wblk_np[1, :, 64:128] = w_attn[:, va : va + 64]
        bblk_np[1, 0:64] = b_attn[ka : ka + 64]
        # Bb = [vB | kB]
        wblk_np[2, :, 0:64] = w_attn[:, vb : vb + 64]
        wblk_np[2, :, 64:128] = w_attn[:, kb_ : kb_ + 64]
        bblk_np[2, 64:128] = b_attn[kb_ : kb_ + 64]

        # packed parity x columns: tiles g = 2m + delta, m = 0..15
        pos = np.concatenate([
            np.arange((2 * m + delta) * 128, (2 * m + delta) * 128 + 128)
            for m in range(NTB)])
        xbT_np = np.ascontiguousarray(xTc[:, pos])

        # wp_sb rows 0:64 = WpA rows, 64:128 = WpB rows, cols 0:768
        wp_in = np.zeros((128, C), np.float32)
        wp_in[0:64, :] = w_proj[hA * 64 : hA * 64 + 64, :]
        wp_in[64:128, :] = w_proj[hB * 64 : hB * 64 + 64, :]

        in_maps.append({
            "onesd": onesd_np,
            "xT": xTc, "xbT": xbT_np, "wblk": wblk_np, "bblk": bblk_np,
            "wp": wp_in, "tri": tri_np,
            "trib1": mask_block(delta, 256),
            "trib2": mask_block(2 + delta, 512),
            "identb": identb_np, "ident": ident_np,
        })
    return in_maps


def kernel(x, w_attn, b_attn, w_proj, b_proj, _trace=False):
    x = np.asarray(x, np.float32)
    w_attn = np.asarray(w_attn, np.float32)
    b_attn = np.asarray(b_attn, np.float32)
    w_proj = np.asarray(w_proj, np.float32)
    b_proj = np.asarray(b_proj, np.float32)

    if "nc" not in _CACHE:
        _CACHE["nc"] = build_program()
    nc = _CACHE["nc"]
    in_maps = _make_inputs(x, w_attn, b_attn, w_proj)
    res = run_bass_kernel_spmd(nc, in_maps, core_ids=list(range(8)),
                               trace=_trace)
    total = np.zeros((T, C), np.float32)
    for c in range(8):
        total += res.results[c]["outa"]
    for p in range(4):
        i, j = 2 * p, 2 * p + 1
        den = (res.results[i]["denb"] + res.results[j]["denb"]).reshape(T, 1)
        total += (res.results[i]["outb"] + res.results[j]["outb"]) / den
    total += b_proj[None, :] + (b_attn[2 * C :] @ w_proj)[None, :]
    if _trace:
        _CACHE["last_result"] = res
    return total.reshape(1, T, C)


# revision 14
# speedup vs baseline: 1.3433x; 1.3433x over previous
"""Causal self-attention (B=1, T=4096, C=768, H=12, D=64) on 8 NeuronCores.

Tensor-parallel, 1.5 heads per core: core c owns full head c ("A") plus
half of shared head 8+c//2 ("B") — the even (c%2==0) or odd k-tiles of
every supertile, so both halves have identical program shape (uniform
SPMD).  The host merges the B halves: out += (rawWp_i + rawWp_j) /
(den_i + den_j), which is exact because the per-q softmax division
commutes with the right-multiplication by w_proj.

Per core, software-pipelined:
  phase 1 (8 steps, x^T + parity-packed xB^T streamed through SBUF rings):
    P  = [qA|qB]^T x^T + b      [128,T]   (rows 0:64 qA*0.125, 64:128 qB)
    K' = [kA|vA]^T x^T + b      [128,512] (kA rows persist; vA rows
                                 PE-transposed into v slots [T,64]+ones)
    Bb = [vB|kB]^T xB^T + b     [128,256] (kB rows persist @64:128, so the
                                 s^T matmul bases match qB; vB transposed)
  phase 2 (A(J) and B(J) generators interleaved):
    s^T[k,q] = k.q in PSUM [128,1024] chunks (2 k-tiles); causal masks are
       additive bf16 matmuls — static tri for A, per-core mask data for B
       (B's diagonal offset depends on the core's k-parity)
    p^T = exp(s^T) on ScalarE -> SBUF
    yt[65,q] = [v|1]^T p^T accumulated in PSUM (row 64 = denominators)
    A: recip(den) on DVE, PE-transpose [1,128]->[128,1] per q-tile, then
       normalization rides the proj PSUM->SBUF move as tensor_scalar mult
    B: no normalization — raw yt + den row go out; host divides
  proj per q-tile: outA[q,768] = yA^T WpA (scaled by 1/denA),
                   outB[q,768] = yB_raw^T WpB (raw), both DMA'd out

All matmul operands float32r (full PE rate at N>=256, HW-verified
213ns per 512-col matmul).
"""
import numpy as np
from contextlib import ExitStack

import concourse.bass as bass
import concourse.mybir as mybir
import concourse.tile as tile
from concourse import bacc
from concourse.bass import ts
from concourse.bass_utils import run_bass_kernel_spmd

try:
    import ml_dtypes
    ml_bf16 = ml_dtypes.bfloat16
except ImportError:  # pragma: no cover
    ml_bf16 = np.float32

F32 = mybir.dt.float32
F32R = mybir.dt.float32r
BF16 = mybir.dt.bfloat16
EXP = mybir.ActivationFunctionType.Exp

T, C, H, D = 4096, 768, 12, 64
KC = C // 128          # 6 contraction chunks of 128
TQ = 512               # q supertile width
NJ = T // TQ           # 8 supertiles
NT = T // 128          # 32 k-tiles (head A); head B has NT//2 packed tiles
NTB = NT // 2
TB = T // 2            # 2048 packed B columns
CH = 2                 # k-tiles per exp chunk
NEG = -60.0            # additive mask value (exp(-60) ~ 0)
NSLOT = NT + NTB       # v slots: A tiles then B tiles

_CACHE = {}


def build_program(reps=1):
    nc = bacc.Bacc()
    xT = nc.dram_tensor("xT", [C, T], F32R, kind="ExternalInput")
    xbT = nc.dram_tensor("xbT", [C, TB], F32R, kind="ExternalInput")
    # weight blocks: [block P | block K' | block Bb] along last axis
    wblk = nc.dram_tensor("wblk", [3, C, 128], F32R, kind="ExternalInput")
    bblk = nc.dram_tensor("bblk", [3, 128], F32, kind="ExternalInput")
    wp = nc.dram_tensor("wp", [128, C], F32R, kind="ExternalInput")
    tri = nc.dram_tensor("tri", [128, 128], BF16, kind="ExternalInput")
    trib1 = nc.dram_tensor("trib1", [128, 256], BF16, kind="ExternalInput")
    trib2 = nc.dram_tensor("trib2", [128, 512], BF16, kind="ExternalInput")
    identb = nc.dram_tensor("identb", [128, 128], BF16, kind="ExternalInput")
    ident = nc.dram_tensor("ident", [128, 128], F32R, kind="ExternalInput")
    onesd = nc.dram_tensor("onesd", [NSLOT * 2], F32R, kind="ExternalInput")
    outa = nc.dram_tensor("outa", [T, C], F32, kind="ExternalOutput")
    outb = nc.dram_tensor("outb", [T, C], F32, kind="ExternalOutput")
    denb = nc.dram_tensor("denb", [NJ, TQ], F32, kind="ExternalOutput")

    with ExitStack() as ctx:
        tc = ctx.enter_context(tile.TileContext(nc))
        singles = ctx.enter_context(tc.tile_pool(name="singles", bufs=1))
        ring = ctx.enter_context(tc.tile_pool(name="ring", bufs=12))
        ringb = ctx.enter_context(tc.tile_pool(name="ringb", bufs=12))
        vring = ctx.enter_context(tc.tile_pool(name="vring", bufs=4))
        sb_p = ctx.enter_context(tc.tile_pool(name="sb_p", bufs=4))
        sb_y = ctx.enter_context(tc.tile_pool(name="sb_y", bufs=2))
        sb_r = ctx.enter_context(tc.tile_pool(name="sb_r", bufs=2))
        sb_o = ctx.enter_context(tc.tile_pool(name="sb_o", bufs=3))
        ps_qk = ctx.enter_context(tc.tile_pool(name="ps_qk", bufs=2, space="PSUM"))
        ps_s = ctx.enter_context(tc.tile_pool(name="ps_s", bufs=2, space="PSUM"))
        ps_yt = ctx.enter_context(tc.tile_pool(name="ps_yt", bufs=2, space="PSUM"))

        # ---- constants / weights ----
        wblk_sb = singles.tile([128, 3, KC, 128], F32R)
        nc.sync.dma_start(
            wblk_sb, wblk.rearrange("b (kc p) m -> p b kc m", p=128))
        bblk_sb = singles.tile([128, 3], F32)
        nc.sync.dma_start(bblk_sb, bblk.rearrange("b p -> p b"))
        wp_sb = singles.tile([128, C], F32R)
        tri_sb = singles.tile([128, 128], BF16)
        nc.sync.dma_start(tri_sb, tri[:, :])
        trib1_sb = singles.tile([128, 256], BF16)
        nc.sync.dma_start(trib1_sb, trib1[:, :])
        trib2_sb = singles.tile([128, 512], BF16)
        nc.sync.dma_start(trib2_sb, trib2[:, :])
        identb_sb = singles.tile([128, 128], BF16)
        nc.sync.dma_start(identb_sb, identb[:, :])
        ident_sb = singles.tile([128, 128], F32R)
        nc.sync.dma_start(ident_sb, ident[:, :])
        v_sb = singles.tile([128, NSLOT * 66], F32R)

        # persistent state
        qkP = singles.tile([128, T], F32R)    # rows 0:64 qA, 64:128 qB
        kA = singles.tile([128, T], F32R)     # rows 0:64 kA (64:128 unused)
        kB = singles.tile([128, TB], F32R)    # rows 64:128 kB (packed)
        yfin = singles.tile([128, T], F32R)   # 0:64 yA norm, 64:128 yB raw
        recden = singles.tile([128, NJ * 4], F32)  # 1/denA cols per q-tile

        def vslot(i):
            return i * 66

        def qkv_step(tc_i):
            """Stream x/xB column slices; compute P,K',Bb; transpose v."""
            xs, xbs = [], []
            for kc in range(KC):
                x_sl = ring.tile([128, TQ], F32R, tag="xr", name="x_sl")
                nc.sync.dma_start(x_sl, xT[ts(kc, 128), ts(tc_i, TQ)])
                xs.append(x_sl)
                xb_sl = ringb.tile([128, 256], F32R, tag="xbr", name="xb_sl")
                nc.sync.dma_start(xb_sl, xbT[ts(kc, 128), ts(tc_i, 256)])
                xbs.append(xb_sl)
            # P block
            psP = ps_qk.tile([128, TQ], F32, tag="qk", name="psP")
            for kc in range(KC):
                nc.tensor.matmul(
                    psP, lhsT=wblk_sb[:, 0, kc, :], rhs=xs[kc],
                    start=(kc == 0), stop=(kc == KC - 1))
            nc.vector.tensor_scalar_add(
                qkP[:, ts(tc_i, TQ)], psP, bblk_sb[:, 0:1])
            # K' block (kA | vA)
            psK = ps_qk.tile([128, TQ], F32, tag="qk", name="psK")
            for kc in range(KC):
                nc.tensor.matmul(
                    psK, lhsT=wblk_sb[:, 1, kc, :], rhs=xs[kc],
                    start=(kc == 0), stop=(kc == KC - 1))
            nc.vector.tensor_scalar_add(
                kA[:, ts(tc_i, TQ)], psK, bblk_sb[:, 1:2])
            # Bb block (vB | kB), 256 cols
            psB = ps_qk.tile([128, 256], F32, tag="qk", name="psB")
            for kc in range(KC):
                nc.tensor.matmul(
                    psB, lhsT=wblk_sb[:, 2, kc, :], rhs=xbs[kc],
                    start=(kc == 0), stop=(kc == KC - 1))
            vbt = vring.tile([128, 256], F32R, tag="vbt", name="vbt")
            nc.vector.tensor_scalar_add(vbt, psB, bblk_sb[:, 2:3])
            nc.sync.dma_start(kB[64:128, ts(tc_i, 256)], vbt[64:128, :])
            # vA transposes: global tiles 4t..4t+3 from kA rows 64:128
            for il in range(4):
                i = 4 * tc_i + il
                tp = ps_qk.tile([128, 64], F32R, tag="qk", name="tpA")
                nc.tensor.transpose(
                    tp, kA[64:128, tc_i * TQ + il * 128 :
                           tc_i * TQ + (il + 1) * 128],
                    ident_sb[64:128, 64:128])
                nc.vector.tensor_copy(
                    v_sb[:, vslot(i) : vslot(i) + 64], tp)
            # vB transposes: packed tiles 2t, 2t+1 from vbt rows 0:64
            for il in range(2):
                m = 2 * tc_i + il
                tp2 = ps_qk.tile([128, 64], F32R, tag="qk", name="tpB")
                nc.tensor.transpose(
                    tp2, vbt[0:64, ts(il, 128)], ident_sb[0:64, 0:64])
                nc.vector.tensor_copy(
                    v_sb[:, vslot(NT + m) : vslot(NT + m) + 64], tp2)

        def att_gen(h, J):
            """h=0: full head A over 4J+4 k-tiles; h=1: parity half head B
            over 2J+2 packed tiles."""
            nkt = 4 * J + 4 if h == 0 else 2 * J + 2
            chunks = [list(range(nkt))[i : i + CH] for i in range(0, nkt, CH)]
            yt = ps_yt.tile([128, TQ], F32, tag="yt", name="yt")
            state = {"first": True}

            def emit_s(ch_tiles):
                st = ps_s.tile([128, CH * TQ], F32, tag="st", name="st")
                for j, i in enumerate(ch_tiles):
                    if h == 0:
                        d = i - 4 * J
                        c0 = d * 128 if d > 0 else 0
                        nc.tensor.matmul(
                            st[:, j * TQ + c0 : (j + 1) * TQ],
                            lhsT=kA[0:64, ts(i, 128)],
                            rhs=qkP[0:64, J * TQ + c0 : (J + 1) * TQ],
                            start=True, stop=(d < 0))
                        if d >= 0:
                            nc.tensor.matmul(
                                st[:, j * TQ + d * 128 : j * TQ + (d + 1) * 128],
                                lhsT=tri_sb, rhs=identb_sb,
                                start=False, stop=True, skip_group_check=True)
                    else:
                        diag = i >= 2 * J
                        nc.tensor.matmul(
                            st[:, ts(j, TQ)], lhsT=kB[64:128, ts(i, 128)],
                            rhs=qkP[64:128, ts(J, TQ)],
                            start=True, stop=not diag)
                        if diag:
                            # local tile 2J -> mask cols 0:256 (trib1),
                            # local tile 2J+1 -> mask cols 0:512 (trib2)
                            if i == 2 * J:
                                nc.tensor.matmul(
                                    st[:, j * TQ : j * TQ + 256],
                                    lhsT=identb_sb, rhs=trib1_sb,
                                    start=False, stop=True,
                                    skip_group_check=True)
                            else:
                                nc.tensor.matmul(
                                    st[:, j * TQ : j * TQ + 512],
                                    lhsT=identb_sb, rhs=trib2_sb,
                                    start=False, stop=True,
                                    skip_group_check=True)
                pt = sb_p.tile([128, CH * TQ], F32R, tag="pt", name="pt")
                n = len(ch_tiles) * TQ
                nc.scalar.activation(pt[:, :n], st[:, :n], EXP)
                return pt

            def emit_pv(ch_tiles, pt):
                for j, i in enumerate(ch_tiles):
                    slot = i if h == 0 else NT + i
                    if h == 0:
                        d = i - 4 * J
                        q0 = d * 128 if d > 0 else 0
                    else:
                        q0 = 0
                    nc.tensor.matmul(
                        yt[0:66, q0:TQ],
                        lhsT=v_sb[:, vslot(slot) : vslot(slot) + 66],
                        rhs=pt[:, j * TQ + q0 : (j + 1) * TQ],
                        start=state["first"], stop=(i == nkt - 1),
                        skip_group_check=True)
                    state["first"] = False

            pts = []
            for ci in range(len(chunks) + 1):
                if ci < len(chunks):
                    pts.append(emit_s(chunks[ci]))
                if ci >= 1:
                    emit_pv(chunks[ci - 1], pts[ci - 1])
                yield

            if h == 0:
                # yA + 1/denA: recip the den row, PE-transpose per q-tile
                # (2-row block: fp32r matmul dst innermost count must be even)
                nc.vector.tensor_copy(yfin[0:64, ts(J, TQ)], yt[0:64])
                dr = sb_r.tile([2, TQ], F32R, tag="dr", name="dr")
                with nc.allow_low_precision(reason="fp32r for PE"):
                    nc.vector.reciprocal(dr, yt[64:66, :])
                for qt in range(4):
                    tpd = ps_qk.tile([128, 64], F32R, tag="qk", name="tpd")
                    nc.tensor.transpose(
                        tpd[:, 0:2], dr[0:2, ts(qt, 128)], ident_sb[0:2, 0:2])
                    nc.vector.tensor_copy(
                        recden[:, J * 4 + qt : J * 4 + qt + 1], tpd[:, 0:1])
            else:
                # raw yB + raw den out
                nc.vector.tensor_copy(yfin[64:128, ts(J, TQ)], yt[0:64])
                db = sb_r.tile([1, TQ], F32, tag="db", name="db")
                nc.vector.tensor_copy(db, yt[64:65, :])
                nc.sync.dma_start(denb[J, :], db[0:1, :])

        def proj_step(J):
            for qt in range(4):
                q0 = J * TQ + qt * 128
                rd = recden[:, J * 4 + qt : J * 4 + qt + 1]
                oba = sb_o.tile([128, C], F32, tag="oba", name="oba")
                obb = sb_o.tile([128, C], F32, tag="obb", name="obb")
                for (lo, hi) in ((0, 512), (512, 768)):
                    ppa = ps_qk.tile([128, hi - lo], F32, tag="qk", name="ppa")
                    nc.tensor.matmul(ppa, lhsT=yfin[0:64, q0 : q0 + 128],
                                     rhs=wp_sb[0:64, lo:hi],
                                     start=True, stop=True)
                    nc.vector.tensor_scalar_mul(oba[:, lo:hi], ppa, rd)
                    ppb = ps_qk.tile([128, hi - lo], F32, tag="qk", name="ppb")
                    nc.tensor.matmul(ppb, lhsT=yfin[64:128, q0 : q0 + 128],
                                     rhs=wp_sb[64:128, lo:hi],
                                     start=True, stop=True)
                    nc.vector.tensor_copy(obb[:, lo:hi], ppb)
                nc.sync.dma_start(outa[q0 : q0 + 128, :], oba)
                nc.sync.dma_start(outb[q0 : q0 + 128, :], obb)

        def drive(*gens):
            gl = list(gens)
            while gl:
                for g in list(gl):
                    try:
                        next(g)
                    except StopIteration:
                        gl.remove(g)

        for rep in range(reps):
            a_done, b_done, projected = set(), set(), set()

            def flush_proj():
                for J in range(NJ):
                    if J in a_done and J in b_done and J not in projected:
                        proj_step(J)
                        projected.add(J)

            for t in range(NJ):
                qkv_step(t)
                if t == 0 and rep == 0:
                    nc.sync.dma_start(wp_sb, wp[:, :])
                    ones_view = bass.AP(
                        tensor=v_sb.tensor, offset=v_sb.offset + 64,
                        ap=[list(p) for p in v_sb.ap[:1]] + [[66, NSLOT],
                                                             [1, 2]])
                    nc.sync.dma_start(
                        ones_view,
                        onesd[:][0 : NSLOT * 2].partition_broadcast(128))
                if 1 <= t <= 6:            # A(0..2)/B(0..2) ride phase 1
                    if t % 2 == 1:
                        drive(att_gen(0, t // 2))
                        a_done.add(t // 2)
                    else:
                        drive(att_gen(1, t // 2 - 1))
                        b_done.add(t // 2 - 1)
            for s in range(5):
                drive(att_gen(0, s + 3), att_gen(1, s + 3))
                a_done.add(s + 3)
                b_done.add(s + 3)
                flush_proj()
            flush_proj()

    if not nc.is_finalized():
        nc.finalize()
    return nc


def _make_inputs(x, w_attn, b_attn, w_proj):
    """Build the 8 per-core input maps from full inputs."""
    xTc = np.ascontiguousarray(x.reshape(T, C).T).astype(np.float32)
    tri_np = np.where(np.arange(128)[:, None] >= np.arange(128)[None, :],
                      0.0, NEG).astype(ml_bf16)
    identb_np = np.eye(128, dtype=np.float32).astype(ml_bf16)
    ident_np = np.eye(128, dtype=np.float32)
    onesd_np = np.ones((NSLOT * 2,), np.float32)

    def mask_block(d, width):
        """[128, width] additive mask for a diag k-tile at column offset d."""
        m = np.zeros((128, width), np.float32)
        m[:, : d * 128] = NEG
        if d * 128 < width:
            # applied untransposed (rhs operand): keep k_row <= q_col
            blk = m[:, d * 128 : (d + 1) * 128]
            blk[:] = np.where(
                np.arange(128)[:, None] <= np.arange(128)[None, :], 0.0, NEG)
        return m.astype(ml_bf16)

    in_maps = []
    for c in range(8):
        hA = c
        hB = 8 + c // 2
        delta = c % 2
        qa, ka, va = hA * 64, C + hA * 64, 2 * C + hA * 64
        qb, kb_, vb = hB * 64, C + hB * 64, 2 * C + hB * 64

        wblk_np = np.zeros((3, C, 128), np.float32)
        bblk_np = np.zeros((3, 128), np.float32)
        # P = [qA*0.125 | qB*0.125]
        wblk_np[0, :, 0:64] = w_attn[:, qa : qa + 64] * 0.125
        wblk_np[0, :, 64:128] = w_attn[:, qb : qb + 64] * 0.125
        bblk_np[0, 0:64] = b_attn[qa : qa + 64] * 0.125
        bblk_np[0, 64:128] = b_attn[qb : qb + 64] * 0.125
        # K' = [kA | vA]
        wblk_np[1, :, 0:64] = w_attn[:, ka : ka + 64]
        wblk_np[1, :, 64:128] = w_attn[:, va : va + 64]
        bblk_np[1, 0:64] = b_attn[ka : ka + 64]
        # Bb = [vB | kB]
        wblk_np[2, :, 0:64] = w_attn[:, vb : vb + 64]
        wblk_np[2, :, 64:128] = w_attn[:, kb_ : kb_ + 64]
        bblk_np[2, 64:128] = b_attn[kb_ : kb_ + 64]

        # packed parity x columns: tiles g = 2m + delta, m = 0..15
        pos = np.concatenate([
            np.arange((2 * m + delta) * 128, (2 * m + delta) * 128 + 128)
            for m in range(NTB)])
        xbT_np = np.ascontiguousarray(xTc[:, pos])

        # wp_sb rows 0:64 = WpA rows, 64:128 = WpB rows, cols 0:768
        wp_in = np.zeros((128, C), np.float32)
        wp_in[0:64, :] = w_proj[hA * 64 : hA * 64 + 64, :]
        wp_in[64:128, :] = w_proj[hB * 64 : hB * 64 + 64, :]

        in_maps.append({
            "onesd": onesd_np,
            "xT": xTc, "xbT": xbT_np, "wblk": wblk_np, "bblk": bblk_np,
            "wp": wp_in, "tri": tri_np,
            "trib1": mask_block(delta, 256),
            "trib2": mask_block(2 + delta, 512),
            "identb": identb_np, "ident": ident_np,
        })
    return in_maps


def kernel(x, w_attn, b_attn, w_proj, b_proj, _trace=False):
    x = np.asarray(x, np.float32)
    w_attn = np.asarray(w_attn, np.float32)
    b_attn = np.asarray(b_attn, np.float32)
    w_proj = np.asarray(w_proj, np.float32)
    b_proj = np.asarray(b_proj, np.float32)

    if "nc" not in _CACHE:
        _CACHE["nc"] = build_program()
    nc = _CACHE["nc"]
    in_maps = _make_inputs(x, w_attn, b_attn, w_proj)
    res = run_bass_kernel_spmd(nc, in_maps, core_ids=list(range(8)),
                               trace=_trace)
    total = np.zeros((T, C), np.float32)
    for c in range(8):
        total += res.results[c]["outa"]
    for p in range(4):
        i, j = 2 * p, 2 * p + 1
        den = (res.results[i]["denb"] + res.results[j]["denb"]).reshape(T, 1)
        total += (res.results[i]["outb"] + res.results[j]["outb"]) / den
    total += b_proj[None, :] + (b_attn[2 * C :] @ w_proj)[None, :]
    if _trace:
        _CACHE["last_result"] = res
    return total.reshape(1, T, C)


# revision 15
# speedup vs baseline: 1.6086x; 1.1975x over previous
"""Causal self-attention (B=1, T=4096, C=768, H=12, D=64) on 8 NeuronCores.

Tensor-parallel, 1.5 heads per core: core c owns full head c ("A") plus
half of shared head 8+c//2 ("B") — the even (c%2==0) or odd k-tiles of
every supertile, so both halves have identical program shape (uniform
SPMD).  The host merges the B halves: out += (rawWp_i + rawWp_j) /
(den_i + den_j), which is exact because the per-q softmax division
commutes with the right-multiplication by w_proj.

Per core, software-pipelined:
  phase 1 (8 steps, x^T + parity-packed xB^T streamed through SBUF rings):
    P  = [qA|qB]^T x^T + b      [128,T]   (rows 0:64 qA*0.125, 64:128 qB)
    K' = [kA|vA]^T x^T + b      [128,512] (kA rows persist; vA rows
                                 PE-transposed into v slots [T,64]+ones)
    Bb = [vB|kB]^T xB^T + b     [128,256] (kB rows persist @64:128, so the
                                 s^T matmul bases match qB; vB transposed)
  phase 2 (A(J) and B(J) generators interleaved):
    s^T[k,q] = k.q in PSUM [128,1024] chunks (2 k-tiles); causal masks are
       additive bf16 matmuls — static tri for A, per-core mask data for B
       (B's diagonal offset depends on the core's k-parity)
    p^T = exp(s^T) on ScalarE -> SBUF
    yt[66,q] = [v|1|1]^T p^T in PSUM (rows 64:66 = dup denominators,
       duplicated so the fp32r PE-transpose gets an even dst count)
    A: recip(den) on DVE, PE-transpose [1,128]->[128,1] per q-tile, then
       normalization rides the proj PSUM->SBUF move as tensor_scalar mult
    B: no normalization — raw yt + den row go out; host divides
  proj per q-tile: outA[q,768] = yA^T WpA (scaled by 1/denA),
                   outB[q,768] = yB_raw^T WpB (raw), both DMA'd out

All matmul operands float32r (full PE rate at N>=256, HW-verified
213ns per 512-col matmul).
"""
import numpy as np
from contextlib import ExitStack

import concourse.bass as bass
import concourse.mybir as mybir
import concourse.tile as tile
from concourse import bacc
from concourse.bass import ts
from concourse.bass_utils import run_bass_kernel_spmd

try:
    import ml_dtypes
    ml_bf16 = ml_dtypes.bfloat16
except ImportError:  # pragma: no cover
    ml_bf16 = np.float32

F32 = mybir.dt.float32
F32R = mybir.dt.float32r
BF16 = mybir.dt.bfloat16
EXP = mybir.ActivationFunctionType.Exp

T, C, H, D = 4096, 768, 12, 64
KC = C // 128          # 6 contraction chunks of 128
TQ = 512               # q supertile width
NJ = T // TQ           # 8 supertiles
NT = T // 128          # 32 k-tiles (head A); head B has NT//2 packed tiles
NTB = NT // 2
TB = T // 2            # 2048 packed B columns
CH = 2                 # k-tiles per exp chunk
NEG = -60.0            # additive mask value (exp(-60) ~ 0)
NSLOT = NT + NTB       # v slots: A tiles then B tiles

_CACHE = {}


def build_program(reps=1):
    nc = bacc.Bacc()
    xT = nc.dram_tensor("xT", [C, T], F32R, kind="ExternalInput")
    xbT = nc.dram_tensor("xbT", [C, TB], F32R, kind="ExternalInput")
    # weight blocks: [block P | block K' | block Bb] along last axis
    wblk = nc.dram_tensor("wblk", [3, C, 128], F32R, kind="ExternalInput")
    bblk = nc.dram_tensor("bblk", [3, 128], F32, kind="ExternalInput")
    wp = nc.dram_tensor("wp", [128, C], F32R, kind="ExternalInput")
    tri = nc.dram_tensor("tri", [128, 128], BF16, kind="ExternalInput")
    trib1 = nc.dram_tensor("trib1", [128, 256], BF16, kind="ExternalInput")
    trib2 = nc.dram_tensor("trib2", [128, 512], BF16, kind="ExternalInput")
    identb = nc.dram_tensor("identb", [128, 128], BF16, kind="ExternalInput")
    ident = nc.dram_tensor("ident", [128, 128], F32R, kind="ExternalInput")
    onesd = nc.dram_tensor("onesd", [NSLOT * 2], F32R, kind="ExternalInput")
    outa = nc.dram_tensor("outa", [T, C], F32, kind="ExternalOutput")
    outb = nc.dram_tensor("outb", [T, C], F32, kind="ExternalOutput")
    denb = nc.dram_tensor("denb", [NJ, TQ], F32, kind="ExternalOutput")

    with ExitStack() as ctx:
        tc = ctx.enter_context(tile.TileContext(nc))
        singles = ctx.enter_context(tc.tile_pool(name="singles", bufs=1))
        ring = ctx.enter_context(tc.tile_pool(name="ring", bufs=12))
        ringb = ctx.enter_context(tc.tile_pool(name="ringb", bufs=12))
        vring = ctx.enter_context(tc.tile_pool(name="vring", bufs=4))
        sb_p = ctx.enter_context(tc.tile_pool(name="sb_p", bufs=4))
        sb_r = ctx.enter_context(tc.tile_pool(name="sb_r", bufs=2))
        sb_o = ctx.enter_context(tc.tile_pool(name="sb_o", bufs=3))
        ps_qk = ctx.enter_context(tc.tile_pool(name="ps_qk", bufs=2, space="PSUM"))
        ps_s = ctx.enter_context(tc.tile_pool(name="ps_s", bufs=2, space="PSUM"))
        ps_yt = ctx.enter_context(tc.tile_pool(name="ps_yt", bufs=2, space="PSUM"))

        # ---- constants / weights ----
        wblk_sb = singles.tile([128, 3, KC, 128], F32R)
        nc.sync.dma_start(
            wblk_sb, wblk.rearrange("b (kc p) m -> p b kc m", p=128))
        bblk_sb = singles.tile([128, 3], F32)
        nc.sync.dma_start(bblk_sb, bblk.rearrange("b p -> p b"))
        wp_sb = singles.tile([128, C], F32R)
        tri_sb = singles.tile([128, 128], BF16)
        nc.sync.dma_start(tri_sb, tri[:, :])
        trib1_sb = singles.tile([128, 256], BF16)
        nc.sync.dma_start(trib1_sb, trib1[:, :])
        trib2_sb = singles.tile([128, 512], BF16)
        nc.sync.dma_start(trib2_sb, trib2[:, :])
        identb_sb = singles.tile([128, 128], BF16)
        nc.sync.dma_start(identb_sb, identb[:, :])
        ident_sb = singles.tile([128, 128], F32R)
        nc.sync.dma_start(ident_sb, ident[:, :])
        v_sb = singles.tile([128, NSLOT * 66], F32R)

        # persistent state
        qkP = singles.tile([128, T], F32R)    # rows 0:64 qA, 64:128 qB
        kA = singles.tile([128, T], F32R)     # rows 0:64 kA (64:128 unused)
        kB = singles.tile([128, TB], F32R)    # rows 64:128 kB (packed)
        yfin = singles.tile([128, T], F32R)   # 0:64 yA norm, 64:128 yB raw
        recden = singles.tile([128, NJ * 4], F32)  # 1/denA cols per q-tile

        def vslot(i):
            return i * 66

        def qkv_step(tc_i):
            """Stream x/xB column slices; compute P,K',Bb; transpose v."""
            xs, xbs = [], []
            for kc in range(KC):
                x_sl = ring.tile([128, TQ], F32R, tag="xr", name="x_sl")
                nc.sync.dma_start(x_sl, xT[ts(kc, 128), ts(tc_i, TQ)])
                xs.append(x_sl)
                xb_sl = ringb.tile([128, 256], F32R, tag="xbr", name="xb_sl")
                nc.sync.dma_start(xb_sl, xbT[ts(kc, 128), ts(tc_i, 256)])
                xbs.append(xb_sl)
            # P block
            psP = ps_qk.tile([128, TQ], F32, tag="qk", name="psP")
            for kc in range(KC):
                nc.tensor.matmul(
                    psP, lhsT=wblk_sb[:, 0, kc, :], rhs=xs[kc],
                    start=(kc == 0), stop=(kc == KC - 1))
            nc.vector.tensor_scalar_add(
                qkP[:, ts(tc_i, TQ)], psP, bblk_sb[:, 0:1])
            # K' block (kA | vA)
            psK = ps_qk.tile([128, TQ], F32, tag="qk", name="psK")
            for kc in range(KC):
                nc.tensor.matmul(
                    psK, lhsT=wblk_sb[:, 1, kc, :], rhs=xs[kc],
                    start=(kc == 0), stop=(kc == KC - 1))
            nc.vector.tensor_scalar_add(
                kA[:, ts(tc_i, TQ)], psK, bblk_sb[:, 1:2])
            # Bb block (vB | kB), 256 cols
            psB = ps_qk.tile([128, 256], F32, tag="qk", name="psB")
            for kc in range(KC):
                nc.tensor.matmul(
                    psB, lhsT=wblk_sb[:, 2, kc, :], rhs=xbs[kc],
                    start=(kc == 0), stop=(kc == KC - 1))
            vbt = vring.tile([128, 256], F32R, tag="vbt", name="vbt")
            nc.vector.tensor_scalar_add(vbt, psB, bblk_sb[:, 2:3])
            nc.sync.dma_start(kB[64:128, ts(tc_i, 256)], vbt[64:128, :])
            # vA transposes: global tiles 4t..4t+3 from kA rows 64:128
            for il in range(4):
                i = 4 * tc_i + il
                tp = ps_qk.tile([128, 64], F32R, tag="qk", name="tpA")
                nc.tensor.transpose(
                    tp, kA[64:128, tc_i * TQ + il * 128 :
                           tc_i * TQ + (il + 1) * 128],
                    ident_sb[64:128, 64:128])
                nc.vector.tensor_copy(
                    v_sb[:, vslot(i) : vslot(i) + 64], tp)
            # vB transposes: packed tiles 2t, 2t+1 from vbt rows 0:64
            for il in range(2):
                m = 2 * tc_i + il
                tp2 = ps_qk.tile([128, 64], F32R, tag="qk", name="tpB")
                nc.tensor.transpose(
                    tp2, vbt[0:64, ts(il, 128)], ident_sb[0:64, 0:64])
                nc.vector.tensor_copy(
                    v_sb[:, vslot(NT + m) : vslot(NT + m) + 64], tp2)

        def att_gen(h, J):
            """h=0: full head A over 4J+4 k-tiles; h=1: parity half head B
            over 2J+2 packed tiles."""
            nkt = 4 * J + 4 if h == 0 else 2 * J + 2
            chunks = [list(range(nkt))[i : i + CH] for i in range(0, nkt, CH)]
            yt = ps_yt.tile([128, TQ], F32, tag="yt", name="yt")
            state = {"first": True}

            def emit_s(ch_tiles):
                st = ps_s.tile([128, CH * TQ], F32, tag="st", name="st")
                for j, i in enumerate(ch_tiles):
                    if h == 0:
                        d = i - 4 * J
                        c0 = d * 128 if d > 0 else 0
                        nc.tensor.matmul(
                            st[:, j * TQ + c0 : (j + 1) * TQ],
                            lhsT=kA[0:64, ts(i, 128)],
                            rhs=qkP[0:64, J * TQ + c0 : (J + 1) * TQ],
                            start=True, stop=(d < 0))
                        if d >= 0:
                            nc.tensor.matmul(
                                st[:, j * TQ + d * 128 : j * TQ + (d + 1) * 128],
                                lhsT=tri_sb, rhs=identb_sb,
                                start=False, stop=True, skip_group_check=True)
                    else:
                        diag = i >= 2 * J
                        nc.tensor.matmul(
                            st[:, ts(j, TQ)], lhsT=kB[64:128, ts(i, 128)],
                            rhs=qkP[64:128, ts(J, TQ)],
                            start=True, stop=not diag)
                        if diag:
                            # local tile 2J -> mask cols 0:256 (trib1),
                            # local tile 2J+1 -> mask cols 0:512 (trib2)
                            if i == 2 * J:
                                nc.tensor.matmul(
                                    st[:, j * TQ : j * TQ + 256],
                                    lhsT=identb_sb, rhs=trib1_sb,
                                    start=False, stop=True,
                                    skip_group_check=True)
                            else:
                                nc.tensor.matmul(
                                    st[:, j * TQ : j * TQ + 512],
                                    lhsT=identb_sb, rhs=trib2_sb,
                                    start=False, stop=True,
                                    skip_group_check=True)
                pt = sb_p.tile([128, CH * TQ], F32R, tag="pt", name="pt")
                n = len(ch_tiles) * TQ
                nc.scalar.activation(pt[:, :n], st[:, :n], EXP)
                return pt

            def emit_pv(ch_tiles, pt):
                for j, i in enumerate(ch_tiles):
                    slot = i if h == 0 else NT + i
                    if h == 0:
                        d = i - 4 * J
                        q0 = d * 128 if d > 0 else 0
                    else:
                        q0 = 0
                    nc.tensor.matmul(
                        yt[0:66, q0:TQ],
                        lhsT=v_sb[:, vslot(slot) : vslot(slot) + 66],
                        rhs=pt[:, j * TQ + q0 : (j + 1) * TQ],
                        start=state["first"], stop=(i == nkt - 1),
                        skip_group_check=True)
                    state["first"] = False

            pts = []
            for ci in range(len(chunks) + 1):
                if ci < len(chunks):
                    pts.append(emit_s(chunks[ci]))
                if ci >= 1:
                    emit_pv(chunks[ci - 1], pts[ci - 1])
                yield

            if h == 0:
                # yA + 1/denA: recip the den row, PE-transpose per q-tile
                # (2-row block: fp32r matmul dst innermost count must be even)
                nc.vector.tensor_copy(yfin[0:64, ts(J, TQ)], yt[0:64])
                dr = sb_r.tile([2, TQ], F32R, tag="dr", name="dr")
                with nc.allow_low_precision(reason="fp32r for PE"):
                    nc.vector.reciprocal(dr, yt[64:66, :])
                for qt in range(4):
                    tpd = ps_qk.tile([128, 64], F32R, tag="qk", name="tpd")
                    nc.tensor.transpose(
                        tpd[:, 0:2], dr[0:2, ts(qt, 128)], ident_sb[0:2, 0:2])
                    nc.vector.tensor_copy(
                        recden[:, J * 4 + qt : J * 4 + qt + 1], tpd[:, 0:1])
            else:
                # raw yB + raw den out
                nc.vector.tensor_copy(yfin[64:128, ts(J, TQ)], yt[0:64])
                db = sb_r.tile([1, TQ], F32, tag="db", name="db")
                nc.vector.tensor_copy(db, yt[64:65, :])
                nc.sync.dma_start(denb[J, :], db[0:1, :])

        def proj_step(J):
            for qt in range(4):
                q0 = J * TQ + qt * 128
                rd = recden[:, J * 4 + qt : J * 4 + qt + 1]
                oba = sb_o.tile([128, C], F32, tag="oba", name="oba")
                obb = sb_o.tile([128, C], F32, tag="obb", name="obb")
                for (lo, hi) in ((0, 512), (512, 768)):
                    ppa = ps_qk.tile([128, hi - lo], F32, tag="qk", name="ppa")
                    nc.tensor.matmul(ppa, lhsT=yfin[0:64, q0 : q0 + 128],
                                     rhs=wp_sb[0:64, lo:hi],
                                     start=True, stop=True)
                    nc.vector.tensor_scalar_mul(oba[:, lo:hi], ppa, rd)
                    ppb = ps_qk.tile([128, hi - lo], F32, tag="qk", name="ppb")
                    nc.tensor.matmul(ppb, lhsT=yfin[64:128, q0 : q0 + 128],
                                     rhs=wp_sb[64:128, lo:hi],
                                     start=True, stop=True)
                    nc.vector.tensor_copy(obb[:, lo:hi], ppb)
                nc.sync.dma_start(outa[q0 : q0 + 128, :], oba)
                nc.sync.dma_start(outb[q0 : q0 + 128, :], obb)

        def drive(*gens):
            gl = list(gens)
            while gl:
                for g in list(gl):
                    try:
                        next(g)
                    except StopIteration:
                        gl.remove(g)

        for rep in range(reps):
            a_done, b_done, projected = set(), set(), set()

            def flush_proj():
                for J in range(NJ):
                    if J in a_done and J in b_done and J not in projected:
                        proj_step(J)
                        projected.add(J)

            for t in range(NJ):
                qkv_step(t)
                if t == 0 and rep == 0:
                    nc.sync.dma_start(wp_sb, wp[:, :])
                    ones_view = bass.AP(
                        tensor=v_sb.tensor, offset=v_sb.offset + 64,
                        ap=[list(p) for p in v_sb.ap[:1]] + [[66, NSLOT],
                                                             [1, 2]])
                    nc.sync.dma_start(
                        ones_view,
                        onesd[:][0 : NSLOT * 2].partition_broadcast(128))
                if 1 <= t <= 6:            # A(0..2)/B(0..2) ride phase 1
                    if t % 2 == 1:
                        drive(att_gen(0, t // 2))
                        a_done.add(t // 2)
                    else:
                        drive(att_gen(1, t // 2 - 1))
                        b_done.add(t // 2 - 1)
            for s in range(5):
                drive(att_gen(0, s + 3), att_gen(1, s + 3))
                a_done.add(s + 3)
                b_done.add(s + 3)
                flush_proj()
            flush_proj()

    if not nc.is_finalized():
        nc.finalize()
    return nc


def _make_inputs(x, w_attn, b_attn, w_proj):
    """Build the 8 per-core input maps from full inputs."""
    xTc = np.ascontiguousarray(x.reshape(T, C).T).astype(np.float32)
    tri_np = np.where(np.arange(128)[:, None] >= np.arange(128)[None, :],
                      0.0, NEG).astype(ml_bf16)
    identb_np = np.eye(128, dtype=np.float32).astype(ml_bf16)
    ident_np = np.eye(128, dtype=np.float32)
    onesd_np = np.ones((NSLOT * 2,), np.float32)

    def mask_block(d, width):
        """[128, width] additive mask for a diag k-tile at column offset d."""
        m = np.zeros((128, width), np.float32)
        m[:, : d * 128] = NEG
        if d * 128 < width:
            # applied untransposed (rhs operand): keep k_row <= q_col
            blk = m[:, d * 128 : (d + 1) * 128]
            blk[:] = np.where(
                np.arange(128)[:, None] <= np.arange(128)[None, :], 0.0, NEG)
        return m.astype(ml_bf16)

    in_maps = []
    for c in range(8):
        hA = c
        hB = 8 + c // 2
        delta = c % 2
        qa, ka, va = hA * 64, C + hA * 64, 2 * C + hA * 64
        qb, kb_, vb = hB * 64, C + hB * 64, 2 * C + hB * 64

        wblk_np = np.zeros((3, C, 128), np.float32)
        bblk_np = np.zeros((3, 128), np.float32)
        # P = [qA*0.125 | qB*0.125]
        wblk_np[0, :, 0:64] = w_attn[:, qa : qa + 64] * 0.125
        wblk_np[0, :, 64:128] = w_attn[:, qb : qb + 64] * 0.125
        bblk_np[0, 0:64] = b_attn[qa : qa + 64] * 0.125
        bblk_np[0, 64:128] = b_attn[qb : qb + 64] * 0.125
        # K' = [kA | vA]
        wblk_np[1, :, 0:64] = w_attn[:, ka : ka + 64]
        wblk_np[1, :, 64:128] = w_attn[:, va : va + 64]
        bblk_np[1, 0:64] = b_attn[ka : ka + 64]
        # Bb = [vB | kB]
        wblk_np[2, :, 0:64] = w_attn[:, vb : vb + 64]
        wblk_np[2, :, 64:128] = w_attn[:, kb_ : kb_ + 64]
        bblk_np[2, 64:128] = b_attn[kb_ : kb_ + 64]

        # packed parity x columns: tiles g = 2m + delta, m = 0..15
        pos = np.concatenate([
            np.arange((2 * m + delta) * 128, (2 * m + delta) * 128 + 128)
            for m in range(NTB)])
        xbT_np = np.ascontiguousarray(xTc[:, pos])

        # wp_sb rows 0:64 = WpA rows, 64:128 = WpB rows, cols 0:768
        wp_in = np.zeros((128, C), np.float32)
        wp_in[0:64, :] = w_proj[hA * 64 : hA * 64 + 64, :]
        wp_in[64:128, :] = w_proj[hB * 64 : hB * 64 + 64, :]

        in_maps.append({
            "onesd": onesd_np,
            "xT": xTc, "xbT": xbT_np, "wblk": wblk_np, "bblk": bblk_np,
            "wp": wp_in, "tri": tri_np,
            "trib1": mask_block(delta, 256),
            "trib2": mask_block(2 + delta, 512),
            "identb": identb_np, "ident": ident_np,
        })
    return in_maps


def kernel(x, w_attn, b_attn, w_proj, b_proj, _trace=False):
    x = np.asarray(x, np.float32)
    w_attn = np.asarray(w_attn, np.float32)
    b_attn = np.asarray(b_attn, np.float32)
    w_proj = np.asarray(w_proj, np.float32)
    b_proj = np.asarray(b_proj, np.float32)

    if "nc" not in _CACHE:
        _CACHE["nc"] = build_program()
    nc = _CACHE["nc"]
    in_maps = _make_inputs(x, w_attn, b_attn, w_proj)
    res = run_bass_kernel_spmd(nc, in_maps, core_ids=list(range(8)),
                               trace=_trace)
    total = np.zeros((T, C), np.float32)
    for c in range(8):
        total += res.results[c]["outa"]
    for p in range(4):
        i, j = 2 * p, 2 * p + 1
        den = (res.results[i]["denb"] + res.results[j]["denb"]).reshape(T, 1)
        total += (res.results[i]["outb"] + res.results[j]["outb"]) / den
    total += b_proj[None, :] + (b_attn[2 * C :] @ w_proj)[None, :]
    if _trace:
        _CACHE["last_result"] = res
    return total.reshape(1, T, C)
